# revision 10
# baseline (speedup 1.0000x reference)
"""Multi-headed attention (B=4, S=2048, D=1024, H=16) on 8 trn2 NeuronCores.

Sharding: core c handles batch b=c//2, head-half hh=c%2 (heads hh*8..hh*8+7).

v3: fused single pipeline, tuned for the two near-equal critical resources:
PE (~1280 matmul slots) and ACT (softmax exp, 256 units x ~1.15us).
 - inputs pre-cast to bf16 on host; whole-row batched DMAs (1 issue per
   weight tensor, 1 issue per x query-tile row-block)
 - K projection ft-major: scores start after 1/4 of it; the rest plus the
   V projection are pumped as guarded background groups into the t=0
   scores stream
 - Q projection for the next section emitted mid-section (kills the
   ACT gap at section boundaries)
 - 3 of 16 exp units per section (t>=1) computed on the idle Vector engine
   with a two-shift Schraudolph exp2 approximation (bitcast int16->bf16,
   mean-centered so it mixes with true-exp chunks; +-1% element ripple)
 - softmax normalization via gpsimd partition_broadcast (no DRAM bounce)
Host: out[b] = core(2b) + core(2b+1) + bo.
"""

import numpy as np
import ml_dtypes

import concourse.tile as tile
from concourse import bacc, mybir
from concourse.bass_utils import run_bass_kernel_spmd

B, S, D, H = 4, 2048, 1024, 16
HD = D // 2          # feature columns per core (8 heads * 64)
KC = D // 128        # 8 contraction chunks over model dim
FT = HD // 128       # 4 feature tiles (head pairs)
ST = S // 512        # 4 query tiles
RT = S // 128        # 16 row tiles / S_k chunks

f32 = mybir.dt.float32
bf16 = mybir.dt.bfloat16
i16 = mybir.dt.int16
MM_DT = bf16
EXP = mybir.ActivationFunctionType.Exp
MULT = mybir.AluOpType.mult
ADD = mybir.AluOpType.add

# two-shift Schraudolph exp2 on DVE: P = bc(i16(SC*s+B1)) + W2*bc(i16(SC*s+B2))
SCHR_SC = float(128.0 * np.log2(np.e) * 0.125)   # fold the 1/8 scores scale
SCHR_B1 = 16256.0 - 134.9
SCHR_B2 = SCHR_B1 + 64.0
SCHR_W2 = 0.70710678
OFF_CC = (1, 4, 6)   # cc slots whose sB exp unit runs on DVE (t>=1)

_CACHED_NC = None
_LAST_IN_MAPS = None


def build_nc():
    nc = bacc.Bacc("TRN2", target_bir_lowering=False, debug=False)

    xq_d = nc.dram_tensor("xq", (D, S), bf16, kind="ExternalInput")
    xk_d = nc.dram_tensor("xk", (D, S), bf16, kind="ExternalInput")
    xv_d = nc.dram_tensor("xv", (D, S), bf16, kind="ExternalInput")
    wq_d = nc.dram_tensor("wq", (D, HD), bf16, kind="ExternalInput")
    wk_d = nc.dram_tensor("wk", (D, HD), bf16, kind="ExternalInput")
    wv_d = nc.dram_tensor("wv", (D, HD), bf16, kind="ExternalInput")
    wo_d = nc.dram_tensor("wo", (HD, D), bf16, kind="ExternalInput")
    bqr_d = nc.dram_tensor("bqr", (128, FT), f32, kind="ExternalInput")
    bkr_d = nc.dram_tensor("bkr", (128, FT), f32, kind="ExternalInput")
    bv_d = nc.dram_tensor("bv", (1, HD), f32, kind="ExternalInput")
    o_d = nc.dram_tensor("o", (S, D), f32, kind="ExternalOutput")

    xq_src = xq_d[:].rearrange("(k p) s -> p k s", p=128)
    xk_src = xk_d[:].rearrange("(k p) s -> p k s", p=128)
    xv_src = xv_d[:].rearrange("(k p) s -> p k s", p=128)

    with tile.TileContext(nc) as tc:
        with (
            tc.tile_pool(name="cpool", bufs=1) as cpool,
            tc.tile_pool(name="big", bufs=1) as big,
            tc.tile_pool(name="xkp", bufs=4) as xkp,
            tc.tile_pool(name="xvp", bufs=2) as xvp,
            tc.tile_pool(name="xqp", bufs=2) as xqp,
            tc.tile_pool(name="qt", bufs=3) as qtp,
            tc.tile_pool(name="ptp", bufs=12) as ptp,
            tc.tile_pool(name="i16p", bufs=4) as i16p,
            tc.tile_pool(name="nrm", bufs=3) as nrm,
            tc.tile_pool(name="rsp", bufs=2) as rsp,
            tc.tile_pool(name="ostage", bufs=3) as ostage,
            tc.tile_pool(name="rsd", bufs=2, space="DRAM") as rsd,
            tc.tile_pool(name="psc", bufs=2, space="PSUM") as psc,
            tc.tile_pool(name="px", bufs=2, space="PSUM") as px,
            tc.tile_pool(name="pq", bufs=2, space="PSUM") as pq,
        ):
            # ---------------- constants / biases ----------------
            onecol_f = cpool.tile([128, 1], f32, name="onecol_f")
            nc.gpsimd.memset(onecol_f[:], 1.0)
            bqr_s = cpool.tile([128, FT], f32, name="bqr_s")
            nc.sync.dma_start(bqr_s[:], bqr_d[:])
            bkr_s = cpool.tile([128, FT], f32, name="bkr_s")
            nc.sync.dma_start(bkr_s[:], bkr_d[:])
            bv_bc = cpool.tile([128, HD], f32, name="bv_bc")
            nc.sync.dma_start(bv_bc[:], bv_d[0:1, :].to_broadcast((128, HD)))

            K = big.tile([128, FT, S], MM_DT, name="Kfm")
            Vs = big.tile([128, RT, 8, 65], MM_DT, name="Vs")
            X = big.tile([128, FT, S], MM_DT, name="Xfm")
            nc.vector.tensor_copy(
                Vs[:, :, :, 64:65],
                onecol_f[:, 0:1].to_broadcast((128, RT, 8, 1)),
            )

            # ---------------- weights: one DMA each ----------------
            wk_s = big.tile([128, KC, HD], MM_DT, name="wk_s")
            wv_s = big.tile([128, KC, HD], MM_DT, name="wv_s")
            wq_s = big.tile([128, KC, HD], MM_DT, name="wq_s")
            wo_s = big.tile([128, FT, D], MM_DT, name="wo_s")
            nc.sync.dma_start(
                wk_s[:], wk_d[:].rearrange("(k p) n -> p k n", p=128))
            nc.gpsimd.dma_start(
                wq_s[:], wq_d[:].rearrange("(k p) n -> p k n", p=128))
            nc.gpsimd.dma_start(
                wv_s[:], wv_d[:].rearrange("(k p) n -> p k n", p=128))
            nc.gpsimd.dma_start(
                wo_s[:], wo_d[:].rearrange("(k p) n -> p k n", p=128))

            # ---------------- x staging (one DMA per row-block) -------
            xk_t = []
            for t in range(ST):
                xt = xkp.tile([128, KC, 512], MM_DT, tag="xk", name="xk")
                nc.sync.dma_start(
                    xt[:], xk_src[:, :, t * 512 : (t + 1) * 512])
                xk_t.append(xt)

            xq_t = {}

            def load_xq(t):
                xt = xqp.tile([128, KC, 512], MM_DT, tag="xq", name="xq")
                nc.gpsimd.dma_start(
                    xt[:], xq_src[:, :, t * 512 : (t + 1) * 512])
                xq_t[t] = xt

            load_xq(0)
            load_xq(1)

            xv_t = {}

            def load_xv(g):
                xt = xvp.tile([128, KC, 512], MM_DT, tag="xv", name="xv")
                nc.gpsimd.dma_start(
                    xt[:], xv_src[:, :, g * 512 : (g + 1) * 512])
                xv_t[g] = xt

            load_xv(0)
            load_xv(1)

            # ---------------- K projection groups ----------------
            def emit_kgroup(ft, t):
                ps = pq.tile([128, 512], f32, tag="pacc", name="pk")
                for kc in range(KC):
                    nc.tensor.matmul(
                        ps[:],
                        wk_s[:, kc, ft * 128 : (ft + 1) * 128],
                        xk_t[t][:, kc, :],
                        start=(kc == 0),
                        stop=(kc == KC - 1),
                    )
                nc.vector.tensor_scalar_add(
                    K[:, ft, t * 512 : (t + 1) * 512],
                    ps[:],
                    bkr_s[:, ft : ft + 1],
                )

            for t in range(ST):
                emit_kgroup(0, t)
            bgk = [(ft, t) for ft in range(1, FT) for t in range(ST)]

            # ---------------- V projection groups ----------------
            vstate = [0]

            def emit_vgroup():
                rt = vstate[0]
                g, rr = rt // 4, rt % 4
                if rr == 0 and g + 2 < 4:
                    load_xv(g + 2)
                ps = pq.tile([128, 512], f32, tag="pacc", name="pv")
                for kc in range(KC):
                    nc.tensor.matmul(
                        ps[:],
                        xv_t[g][:, kc, rr * 128 : (rr + 1) * 128],
                        wv_s[:, kc, :],
                        start=(kc == 0),
                        stop=(kc == KC - 1),
                    )
                nc.vector.tensor_add(
                    Vs[:, rt, :, 0:64],
                    ps[:].rearrange("p (h e) -> p h e", h=8),
                    bv_bc[:].rearrange("p (h e) -> p h e", h=8),
                )
                vstate[0] = rt + 1

            ktoggle = [True]

            def pump_t0(n):
                # alternate K/V groups so AV consumption (gated on V rows)
                # starts early and P-tile buffering stays bounded
                for _ in range(n):
                    if ktoggle[0] and bgk:
                        emit_kgroup(*bgk.pop(0))
                    elif vstate[0] < RT:
                        emit_vgroup()
                    elif bgk:
                        emit_kgroup(*bgk.pop(0))
                    ktoggle[0] = not ktoggle[0]

            # ---------------- deferred output projection ----------------
            bg = []

            def mk_outproj(t2):
                def mk(r2, n):
                    def g():
                        rt = t2 * 4 + r2
                        rsl = slice(rt * 128, (rt + 1) * 128)
                        ps = pq.tile([128, 512], f32, tag="pacc", name="pso")
                        for fc in range(FT):
                            nc.tensor.matmul(
                                ps[:],
                                X[:, fc, rsl],
                                wo_s[:, fc, n * 512 : (n + 1) * 512],
                                start=(fc == 0),
                                stop=(fc == FT - 1),
                            )
                        ot = ostage.tile([128, 512], f32, tag="os", name="os")
                        nc.vector.tensor_copy(ot[:], ps[:])
                        nc.gpsimd.dma_start(
                            o_d[rsl, n * 512 : (n + 1) * 512], ot[:])
                    return g
                return [mk(r2, n) for r2 in range(4) for n in range(2)]

            def pump_bg(n):
                for _ in range(n):
                    if bg:
                        bg.pop(0)()

            # ---------------- normalization ----------------
            def emit_normalize(j2, rsj, tsl2):
                rrh = nrm.tile([128, 512], f32, tag="rr", name="rr")
                nc.vector.reciprocal_approx_fast(rrh[:], rsj[:])
                rd = rsd.tile([2, 512], f32, tag="rd", name="rd")
                for hh in range(2):
                    nc.sync.dma_start(
                        rd[hh : hh + 1, :],
                        rrh[32 * hh : 32 * hh + 1, :])
                bcs = nrm.tile([128, 512], f32, tag="bcs", name="bcs")
                for hh in range(2):
                    pb = 64 * hh
                    nc.sync.dma_start(
                        bcs[pb : pb + 64, :],
                        rd[hh : hh + 1, :].to_broadcast((64, 512)))
                nc.vector.tensor_mul(
                    X[:, j2, tsl2], X[:, j2, tsl2], bcs[:])

            # ---------------- attention ----------------
            def emit_qt(t2, j2):
                qp = pq.tile([128, 512], f32, tag="pacc", name="qp")
                for kc in range(KC):
                    nc.tensor.matmul(
                        qp[:],
                        wq_s[:, kc, j2 * 128 : (j2 + 1) * 128],
                        xq_t[t2][:, kc, :],
                        start=(kc == 0),
                        stop=(kc == KC - 1),
                    )
                Qt = qtp.tile([128, 512], MM_DT, tag="qt", name="qt")
                nc.vector.tensor_scalar_add(
                    Qt[:], qp[:], bqr_s[:, j2 : j2 + 1])
                return Qt

            def emit_av(item, xpA, xpB, j):
                cc, pA, pB = item
                for hf in range(2):
                    kc = 2 * cc + hf
                    nc.tensor.matmul(
                        xpA[:], Vs[:, kc, 2 * j, :], pA[:, hf, :],
                        start=(kc == 0), stop=(kc == RT - 1),
                    )
                    nc.tensor.matmul(
                        xpB[:], Vs[:, kc, 2 * j + 1, :], pB[:, hf, :],
                        start=(kc == 0), stop=(kc == RT - 1),
                    )

            def exp_act(src):
                p = ptp.tile([128, 2, 512], MM_DT, tag="pt", name="p")
                nc.scalar.activation(p[:], src[:], EXP, scale=0.125)
                return p

            def exp_dve(src):
                ia = i16p.tile([128, 2, 512], i16, tag="i16", name="ia")
                nc.vector.tensor_scalar(
                    ia[:], src[:], SCHR_SC, SCHR_B1, MULT, ADD)
                ib = i16p.tile([128, 2, 512], i16, tag="i16", name="ib")
                nc.vector.tensor_scalar(
                    ib[:], src[:], SCHR_SC, SCHR_B2, MULT, ADD)
                p = ptp.tile([128, 2, 512], MM_DT, tag="pt", name="p")
                nc.vector.scalar_tensor_tensor(
                    p[:], ib[:].bitcast(bf16), SCHR_W2, ia[:].bitcast(bf16),
                    MULT, ADD)
                return p

            qt_next = emit_qt(0, 0)
            norm_pending = None
            for t in range(ST):
                tsl = slice(t * 512, (t + 1) * 512)
                for j in range(FT):
                    t0 = (t == 0)
                    Qt = qt_next
                    qt_next = None
                    if norm_pending is not None:
                        emit_normalize(*norm_pending)
                        norm_pending = None
                    if t0 and bgk:
                        # safety: pair row j's K columns must exist
                        while bgk and bgk[0][0] <= j:
                            emit_kgroup(*bgk.pop(0))

                    xpA = px.tile([65, 512], f32, tag="px", name="xpA")
                    xpB = px.tile([65, 512], f32, tag="px", name="xpB")
                    pend = []
                    for cc in range(8):
                        sA = psc.tile([128, 2, 512], f32, tag="sc", name="sA")
                        sB = psc.tile([128, 2, 512], f32, tag="sc", name="sB")
                        for hf in range(2):
                            kc = 2 * cc + hf
                            ksl = slice(kc * 128, (kc + 1) * 128)
                            nc.tensor.matmul(
                                sA[:, hf, :], K[0:64, j, ksl], Qt[0:64, :],
                                start=True, stop=True, tile_position=(0, 0),
                            )
                            nc.tensor.matmul(
                                sB[:, hf, :], K[64:128, j, ksl],
                                Qt[64:128, :],
                                start=True, stop=True, tile_position=(64, 0),
                            )
                        pA = exp_act(sA)
                        if not t0 and cc in OFF_CC:
                            pB = exp_dve(sB)
                        else:
                            pB = exp_act(sB)
                        pend.append((cc, pA, pB))
                        if cc == 5:
                            if j + 1 < FT:
                                qt_next = emit_qt(t, j + 1)
                            elif t + 1 < ST:
                                qt_next = emit_qt(t + 1, 0)
                                if t + 2 < ST:
                                    load_xq(t + 2)
                        if t0:
                            pump_t0(2 if j < 2 else 1)
                            while (pend and
                                   2 * pend[0][0] + 1 < vstate[0] - 2):
                                emit_av(pend.pop(0), xpA, xpB, j)
                        else:
                            if len(pend) > 2:
                                emit_av(pend.pop(0), xpA, xpB, j)
                            if cc in (2, 5, 7):
                                pump_bg(1)
                    for item in pend:
                        if t0:
                            while vstate[0] <= 2 * item[0] + 1:
                                emit_vgroup()
                        emit_av(item, xpA, xpB, j)

                    # drain: unnormalized X and row sums to SBUF
                    nc.vector.tensor_copy(X[0:64, j, tsl], xpA[0:64, :])
                    nc.vector.tensor_copy(X[64:128, j, tsl], xpB[0:64, :])
                    rsj = rsp.tile([128, 512], f32, tag="rs", name="rs")
                    nc.vector.tensor_copy(rsj[0:1, :], xpA[64:65, :])
                    nc.vector.tensor_copy(rsj[32:33, :], xpB[64:65, :])
                    norm_pending = (j, rsj, tsl)

                # end of tile t: queue its output projection for overlap
                if t == ST - 1:
                    if norm_pending is not None:
                        emit_normalize(*norm_pending)
                        norm_pending = None
                    pump_bg(len(bg))
                    for g in mk_outproj(t):
                        g()
                else:
                    bg.extend(mk_outproj(t))

    nc.compile()
    return nc


def kernel(**inputs):
    global _CACHED_NC, _LAST_IN_MAPS
    if _CACHED_NC is None:
        _CACHED_NC = build_nc()
    nc = _CACHED_NC

    bfdt = ml_dtypes.bfloat16
    query = np.asarray(inputs["query"], dtype=np.float32)
    key = np.asarray(inputs["key"], dtype=np.float32)
    value = np.asarray(inputs["value"], dtype=np.float32)
    fc_w = np.asarray(inputs["fc_w"], dtype=np.float32)
    Wq = np.asarray(inputs["Wq"], dtype=np.float32)
    Wk = np.asarray(inputs["Wk"], dtype=np.float32)
    Wv = np.asarray(inputs["Wv"], dtype=np.float32)
    Wo = np.asarray(inputs["Wo"], dtype=np.float32)
    bq = np.asarray(inputs["bq"], dtype=np.float32)
    bk = np.asarray(inputs["bk"], dtype=np.float32)
    bv = np.asarray(inputs["bv"], dtype=np.float32)
    bo = np.asarray(inputs["bo"], dtype=np.float32)

    wq_eff = (fc_w * Wq).astype(bfdt)
    wk_b = Wk.astype(bfdt)
    wv_b = Wv.astype(bfdt)
    wo_b = Wo.astype(bfdt)
    xq_b = [np.ascontiguousarray(query[b].T).astype(bfdt) for b in range(B)]
    xk_b = [np.ascontiguousarray(key[b].T).astype(bfdt) for b in range(B)]
    xv_b = [np.ascontiguousarray(value[b].T).astype(bfdt) for b in range(B)]

    in_maps = []
    for c in range(8):
        b, hh = c // 2, c % 2
        hs = slice(hh * HD, (hh + 1) * HD)
        in_maps.append({
            "xq": xq_b[b],
            "xk": xk_b[b],
            "xv": xv_b[b],
            "wq": np.ascontiguousarray(wq_eff[:, hs]),
            "wk": np.ascontiguousarray(wk_b[:, hs]),
            "wv": np.ascontiguousarray(wv_b[:, hs]),
            "wo": np.ascontiguousarray(wo_b[hs, :]),
            "bqr": np.ascontiguousarray(bq[hs].reshape(FT, 128).T),
            "bkr": np.ascontiguousarray(bk[hs].reshape(FT, 128).T),
            "bv": bv[None, hs],
        })

    _LAST_IN_MAPS = in_maps
    res = run_bass_kernel_spmd(nc, in_maps, core_ids=list(range(8)))

    out = np.empty((B, S, D), dtype=np.float32)
    for b in range(B):
        out[b] = res.results[2 * b]["o"] + res.results[2 * b + 1]["o"] + bo
    return out
